# revision 1
# baseline (speedup 1.0000x reference)
"""Trainium2 Bass kernel for nn_LocalConv2DLayer (fuzzy local conv membership layer).

Math: for input x[B,C,H,W], bounds l_o < r_o forming 32 uniform bins over
[-1,1], the reference computes, per output pixel (b,o,i,j):

    res = sum_{c,kh,kw} (relu(clip(p-l,-1,1)) * relu(clip(r-p,-1,1)) * 4/(r-l)^2)^2

with p = x[b,c,i+kh,j+kw]. Because the bins are disjoint with width
1/16 < 1, the clip at +-1 never affects the product, and each pixel value
falls in exactly one bin. With z = (v - l_0) * scale (scale = 1/(r-l)),
bin index = floor(z), f = frac(z), the per-pixel contribution to its own
bin is val = 16*(f*(1-f))^2 and zero to every other bin.

Kernel structure per core (2 batches, SPMD over 8 cores):
  - layout: partitions = (b_local, h) = 128, free = (c, w) = 192
  - prep: z, f = z mod 1, idx = z - f (fp16), val = (4*relu(f-f^2))^2 (fp16)
  - per output-channel block of 8: e_o = [idx == o] (fp16 0/1),
    msq = e * val, then a banded matmul on PE sums over kh (window rows)
    while PSUM accumulation folds the channel sum; horizontal 5-tap window
    sum via shifted adds; DMA out.
"""

import numpy as np

B, C, O, H, W = 16, 3, 32, 64, 64
KS = 5
NH, NW = H - KS + 1, W - KS + 1  # 60, 60
NCORES = 8
BPC = B // NCORES  # batches per core
P = BPC * H        # 128 partitions = (b_local, h)
M = BPC * NH       # 120 matmul output rows = (b_local, i)
OB = 8             # output channels per block
NBLK = O // OB
FD = C * W         # 192

_CACHE = {}


def _build(scale: float, bias: float):
    import concourse.bass as bass
    import concourse.tile as tile
    from concourse import mybir

    dt = mybir.dt
    Alu = mybir.AluOpType
    Act = mybir.ActivationFunctionType

    nc = bass.Bass()
    # x pre-transposed host-side to [(b h), c, w]; out in kernel-friendly
    # layout [block, (b i), o_local, j], un-transposed host-side.
    blob_d = nc.declare_dram_parameter("blob", [P, FD + M // 2], dt.float32, isOutput=False)
    out_d = nc.declare_dram_parameter("out", [M, O, NW], dt.float32, isOutput=True)

    with tile.TileContext(nc) as tc:
        with (
            tc.tile_pool(name="singles", bufs=1) as singles,
            tc.tile_pool(name="work", bufs=3) as work,
            tc.tile_pool(name="vp", bufs=3) as vp,
            tc.tile_pool(name="ep", bufs=3) as ep,
            tc.tile_pool(name="ps", bufs=3, space="PSUM") as ps,
        ):
            blob_sb = singles.tile([P, FD + M // 2], dt.float32)
            nc.sync.dma_start(out=blob_sb, in_=blob_d[:])
            x_sb = blob_sb[:, 0:FD].rearrange("p (c w) -> p c w", c=C)
            band_sb = blob_sb[:, FD : FD + M // 2].bitcast(dt.float16)

            # PE HAM warmup: harmless matmuls into a scratch PSUM bank
            # while the DVE prep chain runs, so the real matmuls start warm.
            warm_ps = ps.tile([M, 4 * M], dt.float32, tag="warm")
            band_rep = band_sb.rearrange("p (r m) -> p r m", r=1).broadcast_to([P, 4, M])
            for _ in range(24):
                nc.tensor.matmul(warm_ps, lhsT=band_sb, rhs=band_rep, start=True, stop=True)

            MAGIC = 12582912.0  # 1.5 * 2^23; x+M-M == rne(x) for |x| < 2^22

            # prep is all-DVE: pure program order, no cross-engine syncs,
            # and no ScalarE activation-table load on the critical path.
            xf = x_sb.rearrange("p c w -> p (c w)")
            # z2 = z - 0.5 = scale*x + (bias - 0.5); floor(z) = rne(z2) via
            # the magic trick (bin-edge ties land on val == 0, harmless), and
            # fm = f - 0.5 = z2 - idx comes out directly.
            z2 = singles.tile([P, FD], dt.float32)
            nc.vector.tensor_scalar(z2, xf, float(scale), float(bias) - 0.5, op0=Alu.mult, op1=Alu.add)
            t_mag = singles.tile([P, FD], dt.float32)
            nc.vector.tensor_scalar(t_mag, z2, MAGIC, None, op0=Alu.add)
            idx = singles.tile([P, FD], dt.float32)
            nc.vector.tensor_scalar(idx, t_mag, MAGIC, None, op0=Alu.subtract)
            # val = 2^10 * (4*f*(1-f))^2 = (32 - 128*(f-0.5)^2)^2; the 2^10
            # keeps tiny values out of fp16-subnormal range and the band
            # matrix carries the compensating 2^-10. Runs on ScalarE (two
            # Square activations) in parallel with the DVE mask chain.
            fm = singles.tile([P, FD], dt.float32)
            nc.vector.tensor_sub(fm, z2, idx)
            fm2 = singles.tile([P, FD], dt.float32)
            nc.vector.tensor_mul(fm2, fm, fm)
            rq128 = singles.tile([P, FD], dt.float32)
            nc.vector.tensor_scalar(rq128, fm2, -128.0, 32.0, op0=Alu.mult, op1=Alu.add)
            val = singles.tile([P, FD], dt.float16)
            nc.vector.tensor_mul(val, rq128, rq128)

            # split idx into hi (idx>>2) and lo (idx&3): [idx==o] =
            # [hi==o>>2]*[lo==o&3], so 8+4 compares replace 32.
            a_hi = singles.tile([P, FD], dt.float32)
            # offset 0.375 (not 0.5): idx/4 is a quarter-integer, so -0.5
            # would hit exact .5 ties and round-half-even floors wrongly.
            nc.vector.tensor_scalar(a_hi, idx, 0.25, 0.375, op0=Alu.mult, op1=Alu.subtract)
            idxhi = singles.tile([P, FD], dt.float16)
            nc.vector.tensor_scalar(idxhi, a_hi, MAGIC, MAGIC, op0=Alu.add, op1=Alu.subtract)
            hi4 = singles.tile([P, FD], dt.float32)
            nc.vector.tensor_scalar(hi4, idxhi, 4.0, None, op0=Alu.mult)
            idxlo = singles.tile([P, FD], dt.float16)
            nc.vector.tensor_sub(idxlo, idx, hi4)

            NLO, NHI = 4, O // 4
            e_lo = singles.tile([P, NLO, FD], dt.float16)
            for l in range(NLO):
                nc.vector.tensor_scalar(
                    out=e_lo[:, l, :], in0=idxlo,
                    scalar1=float(l), scalar2=0.0,
                    op0=Alu.subtract, op1=Alu.is_equal,
                )
            val_b4 = val.rearrange("p (o f) -> p o f", o=1).broadcast_to([P, NLO, FD])
            vlo = singles.tile([P, NLO, FD], dt.float16)
            nc.vector.tensor_mul(vlo, e_lo, val_b4)

            res_all = singles.tile([M, O, NW], dt.float16)

            HIB = OB // NLO  # hi groups per o-block
            vlo_b = vlo.rearrange("p (h l) f -> p h l f", h=1).broadcast_to([P, HIB, NLO, FD])
            for ob in range(NBLK):
                # per-block ehi tile so block ob doesn't wait on later blocks
                ehi = work.tile([P, HIB, FD], dt.float16, tag="ehi")
                for hl in range(HIB):
                    nc.vector.tensor_scalar(
                        out=ehi[:, hl, :], in0=idxhi,
                        scalar1=float(HIB * ob + hl), scalar2=0.0,
                        op0=Alu.subtract, op1=Alu.is_equal,
                    )
                # msq[o = 8*ob+ol] = vlo[ol&3] * ehi[ol>>2]
                msq = work.tile([P, HIB, NLO, FD], dt.float16, tag="msq")
                ehi_b = (
                    ehi.rearrange("p (h l) f -> p h l f", l=1)
                    .broadcast_to([P, HIB, NLO, FD])
                )
                nc.vector.tensor_mul(msq, vlo_b, ehi_b)
                vps = ps.tile([M, OB, W], dt.float32)
                msq_v = msq.rearrange("p h l (c w) -> p (h l) c w", c=C)
                for c in range(C):
                    nc.tensor.matmul(
                        vps, lhsT=band_sb, rhs=msq_v[:, :, c, :],
                        start=(c == 0), stop=(c == C - 1),
                    )
                v_sb = vp.tile([M, OB, W], dt.float16, tag="v")
                nc.scalar.copy(v_sb, vps)
                E = ep.tile([M, OB, W - 1], dt.float16, tag="E")
                nc.vector.tensor_add(E, v_sb[:, :, 0 : W - 1], v_sb[:, :, 1:W])
                T1 = ep.tile([M, OB, NW], dt.float16, tag="T1")
                nc.vector.tensor_add(T1, E[:, :, 0:NW], E[:, :, 2 : NW + 2])
                res = res_all[:, ob * OB : (ob + 1) * OB, :]
                nc.vector.tensor_add(res, T1, v_sb[:, :, 4 : 4 + NW])
                # half-way + final casting DMAs (fp16 -> fp32): the first
                # overlaps the remaining blocks, only the second is a tail
                if ob == 1:
                    nc.gpsimd.dma_start(out=out_d[:, 0 : 2 * OB, :], in_=res_all[:, 0 : 2 * OB, :])
                if ob == NBLK - 1:
                    nc.gpsimd.dma_start(out=out_d[:, 2 * OB :, :], in_=res_all[:, 2 * OB :, :])
    return nc


def _legalize_multiwaits(bir_json_bytes):
    """Split multi-wait instructions into standalone EventSemaphore waits.

    The walrus codegen in this toolchain accepts at most one inline sync
    wait per compute-engine instruction ("Too many sync wait commands").
    Tile emits joins with several waits; moving the extras onto
    EventSemaphore instructions issued immediately before, on the same
    engine queue, is semantically identical (the engine blocks on them in
    program order before the consumer issues).
    """
    import json

    j = json.loads(bir_json_bytes)
    n_split = 0
    for fn in j["functions"]:
        for blk in fn["blocks"]:
            new_insts = []
            for inst in blk["instructions"]:
                si = inst.get("sync_info") or {}
                waits = si.get("on_wait") or []
                if len(waits) > 1:
                    for k, w in enumerate(waits[:-1]):
                        new_insts.append(
                            {
                                "debug": inst.get("debug"),
                                "engine": inst["engine"],
                                "ins": [],
                                "name": f"{inst['name']}_syncw{k}",
                                "opcode": "EventSemaphore",
                                "outs": [],
                                "sync_info": {"on_update": [], "on_wait": [w]},
                            }
                        )
                    si["on_wait"] = [waits[-1]]
                    n_split += 1
                new_insts.append(inst)
            blk["instructions"] = new_insts
    return json.dumps(j).encode()


def _band_np():
    band = np.zeros((P, M), np.float16)
    for b in range(BPC):
        for h in range(H):
            for i in range(NH):
                if 0 <= h - i < KS:
                    band[b * H + h, b * NH + i] = 2.0 ** -10
    return band


def _get_built(scale, bias):
    key = (round(float(scale), 9), round(float(bias), 9))
    if key not in _CACHE:
        nc = _build(float(scale), float(bias))
        legal = _legalize_multiwaits(nc.to_json_bytes())
        nc.to_json_bytes = lambda: legal
        _CACHE[key] = nc
    return _CACHE[key]


def kernel(x, left_bounds, right_bounds):
    x = np.ascontiguousarray(x, np.float32)
    lb = np.asarray(left_bounds, np.float32).reshape(O, -1)
    rb = np.asarray(right_bounds, np.float32).reshape(O, -1)
    widths = rb[:, 0] - lb[:, 0]
    width = float(widths[0])
    # the kernel's bin decomposition requires uniform contiguous bins
    assert np.allclose(widths, width, rtol=1e-5), "non-uniform bounds unsupported"
    assert np.allclose(lb[1:, 0], rb[:-1, 0], atol=1e-6), "bins must tile the domain"
    scale = 1.0 / width
    bias = -float(lb[0, 0]) * scale

    nc = _get_built(scale, bias)
    band = _band_np()
    band_f32view = np.ascontiguousarray(band).view(np.float32)  # [P, M//2]
    in_maps = []
    for k in range(NCORES):
        xc = x[BPC * k : BPC * (k + 1)]  # [BPC, C, H, W]
        xt = xc.transpose(0, 2, 1, 3).reshape(P, C * W)
        blob = np.ascontiguousarray(np.concatenate([xt, band_f32view], axis=1))
        in_maps.append({"blob": blob})

    from concourse.bass_utils import run_bass_kernel_spmd

    r = run_bass_kernel_spmd(nc, in_maps, list(range(NCORES)))
    global _LAST_RESULT
    _LAST_RESULT = r
    parts = []
    for k in range(NCORES):
        oc = r.results[k]["out"]  # [M, O, NW] = [(b i), o, j]
        oc = oc.reshape(BPC, NH, O, NW).transpose(0, 2, 1, 3)
        parts.append(np.ascontiguousarray(oc))
    out = np.concatenate(parts, axis=0)
    return np.ascontiguousarray(out, np.float32)


_LAST_RESULT = None



# revision 2
# speedup vs baseline: 1.1456x; 1.1456x over previous
"""Trainium2 Bass kernel for nn_LocalConv2DLayer — v2.

Same math as the baseline (see kernel.py docstring): per-pixel bin index
idx = floor(z), z = scale*x + bias, membership val = 2^10*(4*f*(1-f))^2
(fp16), masked per-output-channel and window-summed. v2 restructures for
DVE economy:

  - fused magic-rounding (single tensor_scalar with op0=add/op1=sub)
  - fm2/val squares moved to ScalarE (Square activations; table set is
    shared with the Copy used by the PSUM->SBUF copies, so one load)
  - one-hot-times-val via scalar_tensor_tensor (is_equal, mult) - no
    separate mask tensors
  - super-blocks of 16 output channels: 4 STT masks, 6 matmuls into a
    2-bank PSUM tile, ONE ScalarE copy, 3 window adds, one fp16 HWDGE
    output DMA per super-block
  - PE warmup decoupled from the input DMA (memset zeros), sized to end
    when the first real matmul becomes ready
"""

import numpy as np

B, C, O, H, W = 16, 3, 32, 64, 64
KS = 5
NH, NW = H - KS + 1, W - KS + 1  # 60, 60
NCORES = 8
BPC = B // NCORES
P = BPC * H        # 128
M = BPC * NH       # 120
SB = 16            # output channels per super-block
NSB = O // SB      # 2
NLO = 4
HIB = SB // NLO    # 4 hi-values per super-block
FD = C * W         # 192
NWARM = 12

_CACHE = {}


def _build(scale: float, bias: float):
    import concourse.bass as bass
    import concourse.tile as tile
    from concourse import mybir

    dt = mybir.dt
    Alu = mybir.AluOpType
    Act = mybir.ActivationFunctionType

    nc = bass.Bass()
    blob_d = nc.declare_dram_parameter("blob", [P, FD + M // 2], dt.float32, isOutput=False)
    out_d = nc.declare_dram_parameter("out", [M, O, NW], dt.float16, isOutput=True)

    with tile.TileContext(nc) as tc:
        with (
            tc.tile_pool(name="singles", bufs=1) as singles,
            tc.tile_pool(name="work", bufs=2) as work,
            tc.tile_pool(name="vp", bufs=2) as vp,
            tc.tile_pool(name="ep", bufs=2) as ep,
            tc.tile_pool(name="ps", bufs=2, space="PSUM") as ps,
            tc.tile_pool(name="warm", bufs=1, space="PSUM") as warmp,
        ):
            # PE warmup independent of the input DMA: zeros via memset.
            zt = singles.tile([P, 128], dt.float16)
            nc.gpsimd.memset(zt, 0.0)
            zt_rep = zt.rearrange("p (r m) -> p r m", r=1).broadcast_to([P, 4, 128])
            warm_ps = warmp.tile([P, 512], dt.float32, tag="warm")
            for _ in range(NWARM):
                nc.tensor.matmul(warm_ps, lhsT=zt, rhs=zt_rep, start=True, stop=True)

            blob_sb = singles.tile([P, FD + M // 2], dt.float32)
            nc.sync.dma_start(out=blob_sb, in_=blob_d[:])
            xf = blob_sb[:, 0:FD]
            band_sb = blob_sb[:, FD : FD + M // 2].bitcast(dt.float16)

            MAGIC = 12582912.0  # 1.5 * 2^23; (x+M)-M == rne(x) for |x| < 2^22

            # prep: z2 -> idx -> fm on DVE; fm2/val on ScalarE; hi/lo on DVE
            z2 = singles.tile([P, FD], dt.float32)
            nc.vector.tensor_scalar(z2, xf, float(scale), float(bias) - 0.5, op0=Alu.mult, op1=Alu.add)
            idx = singles.tile([P, FD], dt.float32)
            nc.vector.tensor_scalar(idx, z2, MAGIC, MAGIC, op0=Alu.add, op1=Alu.subtract)
            fm = singles.tile([P, FD], dt.float32)
            nc.vector.tensor_sub(fm, z2, idx)
            # ScalarE: fm2 = fm^2; val = (32 - 128*fm2)^2 in [0, 2^10], fp16
            fm2 = singles.tile([P, FD], dt.float32)
            nc.scalar.activation(fm2, fm, Act.Square)
            # val = (1 - 4*fm2)^2 = (4*f*(1-f))^2 in [0,1]; the band matrix
            # is 1.0 so no compensation factor is needed (bias 1.0 is a
            # pre-registered const AP; 32.0 would need a custom const).
            val = singles.tile([P, FD], dt.float16)
            nc.scalar.activation(val, fm2, Act.Square, bias=1.0, scale=-4.0)
            # hi = idx>>2 (8 values), lo = idx&3
            a_hi = singles.tile([P, FD], dt.float32)
            nc.vector.tensor_scalar(a_hi, idx, 0.25, 0.375, op0=Alu.mult, op1=Alu.subtract)
            idxhi = singles.tile([P, FD], dt.float16)
            nc.vector.tensor_scalar(idxhi, a_hi, MAGIC, MAGIC, op0=Alu.add, op1=Alu.subtract)
            idxlo = singles.tile([P, FD], dt.float16)
            nc.vector.scalar_tensor_tensor(
                idxlo, in0=idxhi, scalar=-4.0, in1=idx, op0=Alu.mult, op1=Alu.add
            )

            # vlo[l] = val * [idxlo == l]
            vlo = singles.tile([P, NLO, FD], dt.float16)
            for l in range(NLO):
                nc.vector.scalar_tensor_tensor(
                    vlo[:, l, :], in0=idxlo, scalar=float(l), in1=val,
                    op0=Alu.is_equal, op1=Alu.mult,
                )

            idxhi_b = idxhi.rearrange("p (l f) -> p l f", l=1).broadcast_to([P, NLO, FD])
            res_all = singles.tile([M, O, NW], dt.float16)

            for sb in range(NSB):
                msq = work.tile([P, HIB, NLO, FD], dt.float16, tag="msq")
                for hl in range(HIB):
                    nc.vector.scalar_tensor_tensor(
                        msq[:, hl], in0=idxhi_b, scalar=float(HIB * sb + hl),
                        in1=vlo, op0=Alu.is_equal, op1=Alu.mult,
                    )
                vps = ps.tile([M, SB, W], dt.float32)
                msq_v = msq.rearrange("p h l (c w) -> p h l c w", c=C)
                for half in range(2):
                    for c in range(C):
                        nc.tensor.matmul(
                            vps[:, 8 * half : 8 * half + 8, :],
                            lhsT=band_sb,
                            rhs=msq_v[:, 2 * half : 2 * half + 2, :, c, :],
                            start=(c == 0), stop=(c == C - 1),
                        )
                v_sb = vp.tile([M, SB, W], dt.float16, tag="v")
                nc.scalar.copy(v_sb, vps)
                E = ep.tile([M, SB, W - 1], dt.float16, tag="E")
                nc.vector.tensor_add(E, v_sb[:, :, 0 : W - 1], v_sb[:, :, 1:W])
                T1 = ep.tile([M, SB, NW], dt.float16, tag="T1")
                nc.vector.tensor_add(T1, E[:, :, 0:NW], E[:, :, 2 : NW + 2])
                res = res_all[:, sb * SB : (sb + 1) * SB, :]
                nc.vector.tensor_add(res, T1, v_sb[:, :, 4 : 4 + NW])
                nc.sync.dma_start(
                    out=out_d[:, sb * SB : (sb + 1) * SB, :], in_=res
                )
    return nc


def _legalize_multiwaits(bir_json_bytes):
    """Split multi-wait instructions into standalone EventSemaphore waits
    (walrus accepts at most one inline sync wait per compute instruction)."""
    import json

    j = json.loads(bir_json_bytes)
    for fn in j["functions"]:
        for blk in fn["blocks"]:
            new_insts = []
            for inst in blk["instructions"]:
                si = inst.get("sync_info") or {}
                waits = si.get("on_wait") or []
                if len(waits) > 1:
                    for k, w in enumerate(waits[:-1]):
                        new_insts.append(
                            {
                                "debug": inst.get("debug"),
                                "engine": inst["engine"],
                                "ins": [],
                                "name": f"{inst['name']}_syncw{k}",
                                "opcode": "EventSemaphore",
                                "outs": [],
                                "sync_info": {"on_update": [], "on_wait": [w]},
                            }
                        )
                    si["on_wait"] = [waits[-1]]
                new_insts.append(inst)
            blk["instructions"] = new_insts
    return json.dumps(j).encode()


def _band_np():
    band = np.zeros((P, M), np.float16)
    for b in range(BPC):
        for h in range(H):
            for i in range(NH):
                if 0 <= h - i < KS:
                    band[b * H + h, b * NH + i] = 1.0
    return band


def _get_built(scale, bias):
    key = (round(float(scale), 9), round(float(bias), 9))
    if key not in _CACHE:
        nc = _build(float(scale), float(bias))
        legal = _legalize_multiwaits(nc.to_json_bytes())
        nc.to_json_bytes = lambda: legal
        _CACHE[key] = nc
    return _CACHE[key]


def _scale_bias(left_bounds, right_bounds):
    lb = np.asarray(left_bounds, np.float32).reshape(O, -1)
    rb = np.asarray(right_bounds, np.float32).reshape(O, -1)
    widths = rb[:, 0] - lb[:, 0]
    width = float(widths[0])
    assert np.allclose(widths, width, rtol=1e-5), "non-uniform bounds unsupported"
    assert np.allclose(lb[1:, 0], rb[:-1, 0], atol=1e-6), "bins must tile the domain"
    scale = 1.0 / width
    bias = -float(lb[0, 0]) * scale
    return scale, bias


def _blob_for_core(x, k, band_f32view):
    xc = x[BPC * k : BPC * (k + 1)]
    xt = xc.transpose(0, 2, 1, 3).reshape(P, C * W)
    return np.ascontiguousarray(np.concatenate([xt, band_f32view], axis=1))


def build_for_sim(x, left_bounds, right_bounds, core=0):
    """Local-sim helper: (in_map, nc, unpack-spec) for one core."""
    x = np.ascontiguousarray(x, np.float32)
    scale, bias = _scale_bias(left_bounds, right_bounds)
    nc = _build(float(scale), float(bias))
    band_f32view = np.ascontiguousarray(_band_np()).view(np.float32)
    in_map = {"blob": _blob_for_core(x, core, band_f32view)}

    def fn(outs):
        oc = outs["out"]
        return (
            oc.reshape(BPC, NH, O, NW).transpose(0, 2, 1, 3).astype(np.float32)
        )

    return in_map, nc, {"outputs": ["out"], "fn": fn}


def kernel(x, left_bounds, right_bounds):
    x = np.ascontiguousarray(x, np.float32)
    scale, bias = _scale_bias(left_bounds, right_bounds)
    nc = _get_built(scale, bias)
    band_f32view = np.ascontiguousarray(_band_np()).view(np.float32)
    in_maps = [{"blob": _blob_for_core(x, k, band_f32view)} for k in range(NCORES)]

    from concourse.bass_utils import run_bass_kernel_spmd

    r = run_bass_kernel_spmd(nc, in_maps, list(range(NCORES)))
    global _LAST_RESULT
    _LAST_RESULT = r
    parts = []
    for k in range(NCORES):
        oc = r.results[k]["out"]
        oc = oc.reshape(BPC, NH, O, NW).transpose(0, 2, 1, 3)
        parts.append(np.ascontiguousarray(oc))
    out = np.concatenate(parts, axis=0)
    return np.ascontiguousarray(out, np.float32)


_LAST_RESULT = None
